# revision 31
# baseline (speedup 1.0000x reference)
"""Distributed Trainium2 Bass kernel for AdaptiveGCN (N=4096, CIN=1024, H=1024, COUT=512, R=10).

Sharding: node dimension split across 8 NeuronCores (512 nodes/core).
Each core owns a 512-column block of the dense adaptive adjacency and the
matching 512 output rows.

v3.1 layout of the math (per core):
  z        = nv1 @ nv2 column block      4x row-tiled K=10 matmuls
  zr       = relu(z)   (DVE, fp16)
  E        = exp(zr)   (scalar, fused row-sum accumulate)  -> AllReduce
  M8       = fp8(128 * r_j * E)          (DVE only; gpsimd port-clashes DVE)
  deg_i    = colsum(r_j E) + 1           4x col-tiled K=128 matmuls + fixup MM
  Y1       = fp8(4 * dinv_i * (xm @ w1)) two fp8 AllGathers (f-halves)
  conv1    = relu((M8^T @ Y1 + 128*Y1_self) * dinv_i / 512 + b1) -> bn1 -> h1
  XW2      = h1 @ w2 (bf16)
  Y2       = fp8(8 * dinv_i * XW2)       one fp8 AllGather (256KB in)
  conv2    = same shape, then mean-pool partials returned per core.

CC stream: dummy warm-up AG -> rowsum AllReduce -> Y1h0 -> Y1h1 -> Y2.
The NEFF entry barrier holds the stream until the slowest rank arrives
(~56us); the dummy absorbs the first-op ncfw cold cost so the AllReduce
starts right at barrier-exit.

kernel(**inputs) takes the FULL unsharded inputs (same keys as
reference.setup_inputs()) and returns the FULL [1, 512] float32 output.
"""

import os
import sys
from contextlib import ExitStack

import numpy as np

for _p in ("/opt/trn_rl_repo", "/root/.axon_site/_ro/trn_rl_repo"):
    if os.path.isdir(_p) and _p not in sys.path:
        sys.path.insert(0, _p)

import concourse.bass as bass
import concourse.bacc as bacc
import concourse.tile as tile
from concourse import mybir
from concourse.bass_utils import run_bass_kernel_spmd
from concourse.masks import make_identity
from concourse.tile_rust import add_dep_helper

F32 = mybir.dt.float32
F16 = mybir.dt.float16
BF16 = mybir.dt.bfloat16
F8 = mybir.dt.float8e4
U8 = mybir.dt.uint8
AF = mybir.ActivationFunctionType
OP = mybir.AluOpType
DR = mybir.MatmulPerfMode.DoubleRow

NCORES = 8
N = 4096
NL = N // NCORES          # 512 nodes per core
CIN = 1024
H = 1024
HQ = H // 2
CO = 512
R = 10
P = 128
JT = N // P               # 32 j-tiles
ET = H // P               # 8
IT = NL // P              # 4 local-node tiles
GT = CO // P              # 4
KC = CIN // P             # 8 cin k-tiles
BN_EPS = 1e-5
SM = 128.0                # fp8 scale on the adjacency block
SY1 = 4.0                 # fp8 scale on Y1
SY2 = 8.0                 # fp8 scale on Y2


def build():
    """Build the SPMD Bass graph (identical on all 8 cores)."""
    nc = bacc.Bacc(None, target_bir_lowering=False, debug=False, num_devices=NCORES)

    # ---- external parameters (per-core shards / replicated) ----
    xT_d = nc.declare_dram_parameter("xT", [CIN, NL], BF16, isOutput=False)
    wmap_d = nc.declare_dram_parameter("wmap", [CIN, H], BF16, isOutput=False)
    w1_d = nc.declare_dram_parameter("w1", [H, H], BF16, isOutput=False)
    w2_d = nc.declare_dram_parameter("w2", [H, CO], BF16, isOutput=False)
    nv1T_d = nc.declare_dram_parameter("nv1T", [R, N], BF16, isOutput=False)
    nv2s_d = nc.declare_dram_parameter("nv2s", [R, NL], BF16, isOutput=False)
    bmap_d = nc.declare_dram_parameter("bmap_t", [P, ET], F32, isOutput=False)
    b1_d = nc.declare_dram_parameter("b1_t", [P, ET], F32, isOutput=False)
    s1_d = nc.declare_dram_parameter("s1_t", [P, ET], F32, isOutput=False)
    t1_d = nc.declare_dram_parameter("t1_t", [P, ET], F32, isOutput=False)
    b2_d = nc.declare_dram_parameter("b2_t", [P, GT], F32, isOutput=False)
    s2_d = nc.declare_dram_parameter("s2_t", [P, GT], F32, isOutput=False)
    t2_d = nc.declare_dram_parameter("t2_t", [P, GT], F32, isOutput=False)
    out_d = nc.declare_dram_parameter("out", [P, GT], F32, isOutput=True)

    # ---- internal DRAM: collective bounce buffers + dinv scratch ----
    rg = [list(range(NCORES))]
    dmy_in = nc.dram_tensor("dmy_in", [1, 8], F32)
    dmy_out = nc.dram_tensor("dmy_out", [NCORES, 8], F32, addr_space="Shared")
    rs_in = nc.dram_tensor("rs_in", [P, JT], F32)
    rs_out = nc.dram_tensor("rs_out", [P, JT], F32, addr_space="Shared")
    y1_in = [nc.dram_tensor(f"y1_in{q}", [NL, HQ], F8) for q in range(2)]
    y1_out = [
        nc.dram_tensor(f"y1_out{q}", [N, HQ], F8, addr_space="Shared")
        for q in range(2)
    ]
    y2_in = nc.dram_tensor("y2_in", [NL, CO], F8)
    y2_out = nc.dram_tensor("y2_out", [N, CO], F8, addr_space="Shared")
    dv_dram = nc.dram_tensor("dv_dram", [NL], F32)

    cc_insts = []

    def collective(in_ap, out_ap, kind="AllGather", op=OP.bypass):
        cc = nc.gpsimd.collective_compute(
            kind, op, replica_groups=rg,
            ins=[in_ap], outs=[out_ap],
        )
        cc_insts.append(cc)
        return cc

    with tile.TileContext(nc) as tc:
        with ExitStack() as ctx:
            # dummy first collective: absorbs entry-barrier + cold ncfw cost
            collective(dmy_in[:], dmy_out[:])

            # ---------- persistent pool ----------
            pp = ctx.enter_context(tc.tile_pool(name="persist", bufs=1))

            # nodevec operands staged at 4 partition offsets for row-tiled
            # K=10 matmuls (tile_position=(32g, 0)); group g owns j-tiles
            # 8g..8g+7
            nv1T4 = pp.tile([P, ET * P], BF16)
            nv2s4 = pp.tile([P, NL], BF16)
            for g in range(4):
                nc.sync.dma_start(
                    nv1T4[32 * g:32 * g + R, :],
                    nv1T_d[:, g * ET * P:(g + 1) * ET * P],
                )
                nc.scalar.dma_start(nv2s4[32 * g:32 * g + R, :], nv2s_d[:])

            # per-k-tile 2D loads spread across queues
            xT_sb = pp.tile([P, KC * NL], BF16)
            wm_sb = pp.tile([P, KC * H], BF16)
            w1_sb = pp.tile([P, KC * H], BF16)
            w2_sb = pp.tile([P, ET * CO], BF16)
            for kt in range(KC):
                nc.sync.dma_start(
                    xT_sb[:, kt * NL:(kt + 1) * NL],
                    xT_d[kt * P:(kt + 1) * P, :],
                )
                nc.gpsimd.dma_start(
                    wm_sb[:, kt * H:(kt + 1) * H],
                    wmap_d[kt * P:(kt + 1) * P, :],
                )
            for kt in range(KC):
                nc.sync.dma_start(
                    w1_sb[:, kt * H:(kt + 1) * H],
                    w1_d[kt * P:(kt + 1) * P, :],
                )
                nc.gpsimd.dma_start(
                    w2_sb[:, kt * CO:(kt + 1) * CO],
                    w2_d[kt * P:(kt + 1) * P, :],
                )

            rs_part = pp.tile([P, JT], F32)
            rowsum_sb = pp.tile([P, JT], F32)
            r_sb = pp.tile([P, JT], F32)
            r_bf = pp.tile([P, JT], BF16)
            r128_sb = pp.tile([P, JT], F32)
            degs_sb = pp.tile([P, NL], F16)
            ones16 = pp.tile([P, 1], F16)
            scr_sb = pp.tile([1, 2], F32)
            dinv_loc = pp.tile([1, NL], F32)
            dinvT = pp.tile([P, IT], F32)
            dinv_rep = pp.tile([P, NL], F32)

            bmap_sb = pp.tile([P, ET], F32)
            b1_sb = pp.tile([P, ET], F32)
            s1_sb = pp.tile([P, ET], F32)
            t1_sb = pp.tile([P, ET], F32)
            b2_sb = pp.tile([P, GT], F32)
            s2_sb = pp.tile([P, GT], F32)
            t2_sb = pp.tile([P, GT], F32)
            for sb, d in (
                (bmap_sb, bmap_d), (b1_sb, b1_d), (s1_sb, s1_d), (t1_sb, t1_d),
                (b2_sb, b2_d), (s2_sb, s2_d), (t2_sb, t2_d),
            ):
                nc.scalar.dma_start(sb[:], d[:])

            M_sb = pp.tile([P, JT * NL], BF16)       # E = exp(relu(z)) block
            M8_sb = pp.tile([P, JT, NL], F8)         # fp8(SM * r_j * E)
            ident_sb = pp.tile([P, P], F32)
            ident8 = pp.tile([P, P], F8)             # SM * I
            diag8 = pp.tile([P, IT, NL], F8)         # self-loop rhs tiles
            make_identity(nc, ident_sb[:])
            nc.vector.tensor_scalar_mul(ident8[:], ident_sb[:], SM)
            nc.gpsimd.memset(diag8[:], 0.0)
            nc.gpsimd.memset(degs_sb[:], 0.0)
            nc.gpsimd.memset(ones16[:], 1.0)
            nc.gpsimd.memset(scr_sb[:], 1.0)
            for it in range(IT):
                nc.vector.tensor_copy(
                    diag8[:, it, it * P:(it + 1) * P], ident8[:]
                )

            xmT_sb = pp.tile([P, ET * NL], BF16)     # relu(x wmap)^T, [f, i]
            xw1bf_sb = pp.tile([P, IT * H], BF16)    # local XW1, [i, f]
            y1q8_sb = pp.tile([P, IT, H], F8)        # fp8(SY1 dinv XW1)
            h1T_sb = pp.tile([P, ET * NL], BF16)     # conv1 out (bn1), [f, i]
            y2q8_sb = pp.tile([P, IT, CO], F8)       # fp8(SY2 dinv XW2)
            h2tmp = pp.tile([P, NL], BF16)
            jtmp = pp.tile([P, P], BF16)
            pool_part = pp.tile([P, GT], F32)

            # exp table preload so the first real Exp skips ACT_TABLE_LOAD
            nc.scalar.activation(scr_sb[:, 0:1], scr_sb[:, 1:2], AF.Exp)

            # ---------- phase A1: z, E, rowsum AR, xmT, XW1 ----------
            with ExitStack() as a1:
                psZ = a1.enter_context(
                    tc.tile_pool(name="psZ", bufs=1, space="PSUM")
                )
                psB = a1.enter_context(
                    tc.tile_pool(name="psB", bufs=3, space="PSUM")
                )
                tmpZ = a1.enter_context(tc.tile_pool(name="tmpZ", bufs=6))

                # z = nv1 @ nv2, 4 row-groups concurrent (K=10); per tile:
                # DVE relu into fp16, scalar Exp with fused rowsum accumulate
                for rr in range(ET):
                    for g in range(4):
                        jt = ET * g + rr
                        zp = psZ.tile([P, NL], F32, tag=f"z{g}", name=f"z{jt}")
                        nc.tensor.matmul(
                            zp[:],
                            nv1T4[32 * g:32 * g + R, rr * P:(rr + 1) * P],
                            nv2s4[32 * g:32 * g + R, :],
                            start=True, stop=True,
                            tile_position=(32 * g, 0),
                        )
                        zr = tmpZ.tile([P, NL], F16, tag="zr", name=f"zr{jt}")
                        nc.vector.tensor_scalar(
                            zr[:], zp[:], 0.0, 0.0, op0=OP.max, op1=OP.add
                        )
                        nc.scalar.activation(
                            M_sb[:, jt * NL:(jt + 1) * NL], zr[:], AF.Exp,
                            accum_out=rs_part[:, jt:jt + 1],
                        )

                # ---- AllReduce softmax row-sum partials ----
                nc.sync.dma_start(rs_in[:], rs_part[:])
                collective(rs_in[:], rs_out[:], kind="AllReduce", op=OP.add)
                # rsqrt table preload during the idle window before dinv
                nc.scalar.activation(
                    scr_sb[:, 0:1], scr_sb[:, 1:2], AF.Abs_reciprocal_sqrt,
                    bias=1.0,
                )
                nc.sync.dma_start(rowsum_sb[:], rs_out[:])
                with nc.allow_low_precision(reason="r feeds a bf16 matmul"):
                    nc.vector.reciprocal(r_bf[:], rowsum_sb[:])
                nc.vector.reciprocal(r_sb[:], rowsum_sb[:])
                nc.vector.tensor_scalar_mul(r128_sb[:], r_sb[:], SM)

                # ---- xmT = relu(wmap^T x^T + b), epilogue on DVE ----
                for et in range(ET):
                    mp = psB.tile([P, NL], F32, tag="mp", name=f"mp{et}")
                    for kt in range(KC):
                        nc.tensor.matmul(
                            mp[:],
                            wm_sb[:, kt * H + et * P: kt * H + (et + 1) * P],
                            xT_sb[:, kt * NL:(kt + 1) * NL],
                            start=(kt == 0), stop=(kt == KC - 1),
                        )
                    nc.vector.tensor_scalar(
                        xmT_sb[:, et * NL:(et + 1) * NL], mp[:],
                        bmap_sb[:, et:et + 1], 0.0,
                        op0=OP.add, op1=OP.max,
                    )

                # ---- XW1 = xm @ w1 (bf16 local copy) ----
                for q in range(2):
                    for it in range(IT):
                        wp = psB.tile([P, HQ], F32, tag="mp",
                                      name=f"wp{q}{it}")
                        for kt in range(KC):
                            nc.tensor.matmul(
                                wp[:],
                                xmT_sb[:, kt * NL + it * P: kt * NL + (it + 1) * P],
                                w1_sb[:, kt * H + q * HQ: kt * H + (q + 1) * HQ],
                                start=(kt == 0), stop=(kt == KC - 1),
                            )
                        nc.vector.tensor_copy(
                            xw1bf_sb[:, it * H + q * HQ: it * H + (q + 1) * HQ],
                            wp[:],
                        )

            # ---------- phase A2: colsum -> dinv -> Y1 cast -> AGs ----------
            with ExitStack() as a2:
                psCS = a2.enter_context(
                    tc.tile_pool(name="psCS", bufs=1, space="PSUM")
                )
                psD = a2.enter_context(
                    tc.tile_pool(name="psD", bufs=1, space="PSUM")
                )
                psJ2 = a2.enter_context(
                    tc.tile_pool(name="psJ2", bufs=1, space="PSUM")
                )
                # keep-warm chain across the AllReduce wait so the colsum
                # matmuls start at the warm clock instead of K=4/8
                jp2 = psJ2.tile([P, P], F32, tag="jk2", name="jk2")
                jprev = None
                for k in range(10):
                    jm = nc.tensor.matmul(
                        jp2[:], ident8[:], ident8[:], start=True, stop=True,
                        skip_group_check=True,
                    )
                    if jprev is not None:
                        add_dep_helper(jm.ins, jprev.ins, True, "warm pace")
                    else:
                        # dummy AG ends right as the AllReduce starts
                        add_dep_helper(jm.ins, cc_insts[0].ins, True,
                                       "warm from dummy end")
                    jprev = nc.vector.tensor_copy(jtmp[:], jp2[:])
                # col-tiled colsum: group cg accumulates j-tiles 8cg..8cg+7
                # into its own bank at psum partition 32cg
                cs = {
                    cg: psCS.tile([P, NL], F32, tag=f"cs{cg}", name=f"cs{cg}")
                    for cg in range(4)
                }
                for cr in range(ET):
                    for cg in range(4):
                        jt = ET * cg + cr
                        nc.tensor.matmul(
                            cs[cg][32 * cg:32 * cg + 1, :],
                            r_bf[:, jt:jt + 1],
                            M_sb[:, jt * NL:(jt + 1) * NL],
                            start=(cr == 0), stop=(cr == ET - 1),
                            tile_position=(0, 32 * cg),
                        )
                # deg fixup in two half-partition matmuls so the second pair
                # of psum-row copies overlaps the first matmul
                dgp = psD.tile([1, NL], F32)
                for cg in range(2):
                    nc.vector.tensor_copy(
                        degs_sb[32 * cg:32 * cg + 1, :],
                        cs[cg][32 * cg:32 * cg + 1, :],
                    )
                nc.tensor.matmul(
                    dgp[:], ones16[0:64, :], degs_sb[0:64, :],
                    start=True, stop=False,
                )
                for cg in range(2, 4):
                    nc.vector.tensor_copy(
                        degs_sb[32 * cg:32 * cg + 1, :],
                        cs[cg][32 * cg:32 * cg + 1, :],
                    )
                nc.tensor.matmul(
                    dgp[:], ones16[64:P, :], degs_sb[64:P, :],
                    start=False, stop=True,
                )
                # dinv = (deg+1)^-1/2 via scalar LUT
                nc.scalar.activation(
                    dinv_loc[:], dgp[:], AF.Abs_reciprocal_sqrt, bias=1.0
                )
                # dinvT via PE transpose (4x [1,128] -> [128,1]); avoids the
                # DRAM round-trip latency on the critical path
                tp = psD.tile([P, IT], F32, tag="dvt", name="dvt")
                for it in range(IT):
                    nc.tensor.transpose(
                        tp[:, it:it + 1],
                        dinv_loc[0:1, it * P:(it + 1) * P],
                        ident_sb[0:1, 0:1],
                    )
                nc.vector.tensor_copy(dinvT[:], tp[:])
                # dinv_rep (epilogue operand, off critical path) via DRAM
                nc.scalar.dma_start(dv_dram[:], dinv_loc[:])
                nc.gpsimd.dma_start(
                    dinv_rep[:], dv_dram[None, :].to_broadcast((P, NL))
                )

                # ---- Y1 = fp8(SY1 * dinv_i * XW1) in q-half order so the
                # first AllGather triggers as early as possible ----
                bounce_insts = []
                for q in range(2):
                    for it in range(IT):
                        nc.vector.tensor_scalar(
                            y1q8_sb[:, it, q * HQ:(q + 1) * HQ],
                            xw1bf_sb[:, it * H + q * HQ: it * H + (q + 1) * HQ],
                            dinvT[:, it:it + 1], SY1,
                            op0=OP.mult, op1=OP.mult,
                        )
                        eng = nc.scalar if it % 2 == 0 else nc.sync
                        bounce_insts.append(eng.dma_start(
                            y1_in[q][it * P:(it + 1) * P, :],
                            y1q8_sb[:, it, q * HQ:(q + 1) * HQ],
                        ))
                    collective(y1_in[q][:], y1_out[q][:])

                # ---- M8 = fp8(SM * r_j * E) on DVE only (gpsimd clashes);
                # held behind the Y1 bounces so the scheduler can't run it
                # ahead of the critical dinv/cast chain ----
                last_bounce = bounce_insts[-1]
                for jt in range(JT):
                    m8 = nc.vector.tensor_scalar_mul(
                        M8_sb[:, jt, :],
                        M_sb[:, jt * NL:(jt + 1) * NL],
                        r128_sb[:, jt:jt + 1],
                    )
                    add_dep_helper(m8.ins, last_bounce.ins, True,
                                   "m8 after y1 bounces")

            # ---------- conv passes (fp8 DoubleRow) ----------
            ENGS2 = [nc.sync, nc.scalar]
            KPE = JT // 2 - 6   # tail kps run mt-major so epilogues overlap

            def conv_pass(mts, slab_pool, ps_pool, y_out, width, off_f,
                          yq8, tagp, epilogue):
                """psum[mt] = M8^T @ Ygathered + SM * Y_self, DoubleRow fp8."""
                psums = {
                    mt: ps_pool.tile([P, NL], F32, tag=f"{tagp}{mt}",
                                     name=f"{tagp}{mt}")
                    for mt in mts
                }
                # self-loop first: doesn't need the gathered slab, so it
                # runs while the AllGather is still in flight
                for mt in mts:
                    for tp in range(IT // 2):
                        nc.tensor.matmul(
                            psums[mt][:],
                            yq8[:, 2 * tp:2 * tp + 2, mt * P:(mt + 1) * P],
                            diag8[:, 2 * tp:2 * tp + 2, :],
                            start=(tp == 0), stop=False,
                            perf_mode=DR,
                        )
                # gathered slab: 32 plain 2D loads (full DMA rate) on the
                # two HWDGE queues
                slab = slab_pool.tile([P, JT, width], F8, tag=f"sl{tagp}",
                                      name=f"sl{tagp}", bufs=1)
                for kt in range(JT):
                    ENGS2[kt % 2].dma_start(
                        slab[:, kt, :], y_out[kt * P:(kt + 1) * P, :]
                    )

                def kmm(kp, mt):
                    fo = mt * P - off_f
                    nc.tensor.matmul(
                        psums[mt][:],
                        slab[:, 2 * kp:2 * kp + 2, fo:fo + P],
                        M8_sb[:, 2 * kp:2 * kp + 2, :],
                        start=False,
                        stop=(kp == JT // 2 - 1),
                        perf_mode=DR,
                    )

                for kp in range(KPE):
                    for mt in mts:
                        kmm(kp, mt)
                # tail: mt-major so each psum finishes early and its
                # epilogue overlaps the next tile's matmuls
                for mt in mts:
                    for kp in range(KPE, JT // 2):
                        kmm(kp, mt)
                    epilogue(mt, psums[mt])

            def mk_epilogue(etmp_pool, b_sb, s_sb, t_sb, inv_scale, tagp,
                            hdst=None, pool_out=None):
                def epilogue(mt, psum):
                    ta = etmp_pool.tile([P, NL], F32, tag=f"ea{tagp}",
                                        name=f"ea{tagp}{mt}")
                    nc.vector.tensor_mul(ta[:], psum[:], dinv_rep[:])
                    tb = etmp_pool.tile([P, NL], F32, tag=f"eb{tagp}",
                                        name=f"eb{tagp}{mt}")
                    if pool_out is not None:
                        # pooled output only: fuse the row-sum into the Relu
                        # and apply the bn scale/shift on the host instead
                        nc.scalar.activation(
                            tb[:], ta[:], AF.Relu,
                            bias=b_sb[:, mt:mt + 1], scale=inv_scale,
                            accum_out=pool_out[:, mt:mt + 1],
                        )
                        return
                    nc.scalar.activation(
                        tb[:], ta[:], AF.Relu,
                        bias=b_sb[:, mt:mt + 1], scale=inv_scale,
                    )
                    nc.vector.tensor_scalar(
                        hdst[:, mt * NL:(mt + 1) * NL], tb[:],
                        s_sb[:, mt:mt + 1], t_sb[:, mt:mt + 1],
                        op0=OP.mult, op1=OP.add,
                    )
                return epilogue

            # conv1: two f-half passes, 8 PSUM banks total
            with ExitStack() as c1:
                ps1 = c1.enter_context(
                    tc.tile_pool(name="ps1", bufs=1, space="PSUM")
                )
                slab1_pool = c1.enter_context(
                    tc.tile_pool(name="slab1", bufs=1)
                )
                etmp = c1.enter_context(tc.tile_pool(name="etmp", bufs=2))
                epi1 = mk_epilogue(etmp, b1_sb, s1_sb, t1_sb,
                                   1.0 / (SM * SY1), "1", hdst=h1T_sb)
                for q in range(2):
                    conv_pass(range(4 * q, 4 * q + 4), slab1_pool, ps1,
                              y1_out[q], HQ, q * HQ, y1q8_sb, f"c1{q}", epi1)

                # XW2 = h1 @ w2 (bf16), contraction mt-outer: each k-tile
                # group becomes ready right after that h1 tile's epilogue,
                # so most of XW2 interleaves into conv1's matmul stream
                # (reuses conv1-q0's freed PSUM banks via the same tags)
                wp2 = {
                    it: ps1.tile([P, CO], F32, tag=f"c10{it}",
                                 name=f"wp2{it}")
                    for it in range(IT)
                }
                for kt in range(ET):
                    for it in range(IT):
                        nc.tensor.matmul(
                            wp2[it][:],
                            h1T_sb[:, kt * NL + it * P: kt * NL + (it + 1) * P],
                            w2_sb[:, kt * CO:(kt + 1) * CO],
                            start=(kt == 0), stop=(kt == ET - 1),
                        )
                y2b_last = None
                for it in range(IT):
                    nc.vector.tensor_scalar(
                        y2q8_sb[:, it, :], wp2[it][:],
                        dinvT[:, it:it + 1], SY2,
                        op0=OP.mult, op1=OP.mult,
                    )
                    eng = nc.scalar if it % 2 == 0 else nc.sync
                    y2b_last = eng.dma_start(
                        y2_in[it * P:(it + 1) * P, :], y2q8_sb[:, it, :]
                    )
                collective(y2_in[:], y2_out[:])

            # conv2
            with ExitStack() as c2:
                ps2 = c2.enter_context(
                    tc.tile_pool(name="ps2", bufs=1, space="PSUM")
                )
                psJ = c2.enter_context(
                    tc.tile_pool(name="psJ", bufs=1, space="PSUM")
                )
                slab2_pool = c2.enter_context(
                    tc.tile_pool(name="slab2", bufs=1)
                )
                etmp2 = c2.enter_context(tc.tile_pool(name="etmp2", bufs=2))

                # keep-warm chain: small self-paced matmuls spanning the Y2
                # AllGather window so the PE clock doesn't re-throttle
                # before conv2's k-loop (each link: MM -> DVE read -> MM...);
                # gated on the Y2 collective trigger so it spans the window
                # instead of front-running it
                jp = psJ.tile([P, P], F32, tag="jk", name="jk")
                prev = None
                for k in range(24):
                    jm = nc.tensor.matmul(
                        jp[:], ident8[:], ident8[:], start=True, stop=True,
                        skip_group_check=True,
                    )
                    if prev is not None:
                        add_dep_helper(jm.ins, prev.ins, True, "warm pace")
                    else:
                        add_dep_helper(jm.ins, y2b_last.ins, True,
                                       "warm from y2 bounce")
                    prev = nc.vector.tensor_copy(jtmp[:], jp[:])

                epi2 = mk_epilogue(etmp2, b2_sb, s2_sb, t2_sb,
                                   1.0 / (SM * SY2), "2",
                                   pool_out=pool_part)
                conv_pass(range(GT), slab2_pool, ps2, y2_out, CO, 0,
                          y2q8_sb, "c2", epi2)

            # per-core pooled partial out; host reduces across cores
            nc.gpsimd.dma_start(out_d[:], pool_part[:])

        # pin the CC stream order: dummy, rs, y1h0, y1h1, y2
        for a, b in zip(cc_insts[1:], cc_insts[:-1]):
            add_dep_helper(a.ins, b.ins, True, "cc stream order")

    nc.compile()
    return nc


_NC_CACHE = {}


def _get_nc():
    if "nc" not in _NC_CACHE:
        _NC_CACHE["nc"] = build()
    return _NC_CACHE["nc"]


def make_in_maps(inputs):
    import ml_dtypes

    f = np.float32
    bf = ml_dtypes.bfloat16
    x = np.asarray(inputs["x"], dtype=f)
    w_map = np.asarray(inputs["w_map"], dtype=f)
    w1 = np.asarray(inputs["w1"], dtype=f)
    w2 = np.asarray(inputs["w2"], dtype=f)
    nv1 = np.asarray(inputs["nv1"], dtype=f)
    nv2 = np.asarray(inputs["nv2"], dtype=f)

    def vec_t(v, nt):
        return np.ascontiguousarray(np.asarray(v, dtype=f).reshape(nt, P).T)

    s1 = (np.asarray(inputs["bn1_g"], f)
          / np.sqrt(np.asarray(inputs["bn1_v"], f) + BN_EPS))
    t1 = np.asarray(inputs["bn1_b"], f) - np.asarray(inputs["bn1_m"], f) * s1
    s2 = (np.asarray(inputs["bn2_g"], f)
          / np.sqrt(np.asarray(inputs["bn2_v"], f) + BN_EPS))
    t2 = np.asarray(inputs["bn2_b"], f) - np.asarray(inputs["bn2_m"], f) * s2

    common = {
        "wmap": np.ascontiguousarray(w_map.astype(bf)),
        "w1": np.ascontiguousarray(w1.astype(bf)),
        "w2": np.ascontiguousarray(w2.astype(bf)),
        "nv1T": np.ascontiguousarray(nv1.T.astype(bf)),
        "bmap_t": vec_t(inputs["b_map"], ET),
        "b1_t": vec_t(inputs["b1"], ET),
        "s1_t": vec_t(s1, ET),
        "t1_t": vec_t(t1, ET),
        "b2_t": vec_t(inputs["b2"], GT),
        "s2_t": vec_t(s2, GT),
        "t2_t": vec_t(t2, GT),
    }
    in_maps = []
    for c in range(NCORES):
        m = dict(common)
        m["xT"] = np.ascontiguousarray(x[c * NL:(c + 1) * NL].T.astype(bf))
        m["nv2s"] = np.ascontiguousarray(nv2[:, c * NL:(c + 1) * NL].astype(bf))
        in_maps.append(m)
    return in_maps


def finish_host(results, inputs):
    """Sum per-core pooled partials, apply bn2 + mean + attention gate."""
    f = np.float32
    pooled_sum = np.zeros(CO, f)
    for res in results:
        arr = np.asarray(res["out"], dtype=f)      # [P, GT], g = t*P + p
        pooled_sum += arr.T.reshape(-1)
    # device pools the pre-bn2 activations; apply bn2 scale/shift here
    s2 = (np.asarray(inputs["bn2_g"], f)
          / np.sqrt(np.asarray(inputs["bn2_v"], f) + BN_EPS))
    t2 = np.asarray(inputs["bn2_b"], f) - np.asarray(inputs["bn2_m"], f) * s2
    pooled = s2 * (pooled_sum / N) + t2
    w_attn = np.asarray(inputs["w_attn"], f).reshape(-1)
    b_attn = np.asarray(inputs["b_attn"], f).reshape(-1)[0]
    z = float(pooled @ w_attn + b_attn)
    attn = 1.0 / (1.0 + np.exp(-z))
    return (pooled * attn)[None, :].astype(f)


def run(inputs, trace=False, tmpdir=None):
    nc = _get_nc()
    in_maps = make_in_maps(inputs)
    res = run_bass_kernel_spmd(
        nc, in_maps, core_ids=list(range(NCORES)), trace=trace, tmpdir=tmpdir
    )
    out = finish_host(res.results, inputs)
    return out, res


def kernel(**inputs):
    out, _ = run(inputs)
    return out


# revision 36
# speedup vs baseline: 1.0589x; 1.0589x over previous
"""Distributed Trainium2 Bass kernel for AdaptiveGCN (N=4096, CIN=1024, H=1024, COUT=512, R=10).

Sharding: node dimension split across 8 NeuronCores (512 nodes/core).
Each core owns a 512-column block of the dense adaptive adjacency and the
matching 512 output rows.

v3.1 layout of the math (per core):
  z        = nv1 @ nv2 column block      4x row-tiled K=10 matmuls
  zr       = relu(z)   (DVE, fp16)
  E        = exp(zr)   (scalar, fused row-sum accumulate)  -> AllReduce
  M8       = fp8(128 * r_j * E)          (DVE only; gpsimd port-clashes DVE)
  deg_i    = colsum(r_j E) + 1           4x col-tiled K=128 matmuls + fixup MM
  Y1       = fp8(4 * dinv_i * (xm @ w1)) two fp8 AllGathers (f-halves)
  conv1    = relu((M8^T @ Y1 + 128*Y1_self) * dinv_i / 512 + b1) -> bn1 -> h1
  XW2      = h1 @ w2 (bf16)
  Y2       = fp8(8 * dinv_i * XW2)       one fp8 AllGather (256KB in)
  conv2    = same shape, then mean-pool partials returned per core.

CC stream: dummy warm-up AG -> rowsum AllReduce -> Y1h0 -> Y1h1 -> Y2.
The NEFF entry barrier holds the stream until the slowest rank arrives
(~56us); the dummy absorbs the first-op ncfw cold cost so the AllReduce
starts right at barrier-exit.

kernel(**inputs) takes the FULL unsharded inputs (same keys as
reference.setup_inputs()) and returns the FULL [1, 512] float32 output.
"""

import os
import sys
from contextlib import ExitStack

import numpy as np

for _p in ("/opt/trn_rl_repo", "/root/.axon_site/_ro/trn_rl_repo"):
    if os.path.isdir(_p) and _p not in sys.path:
        sys.path.insert(0, _p)

import concourse.bass as bass
import concourse.bacc as bacc
import concourse.tile as tile
from concourse import mybir
from concourse.bass_utils import run_bass_kernel_spmd
from concourse.masks import make_identity
from concourse.tile_rust import add_dep_helper

F32 = mybir.dt.float32
F16 = mybir.dt.float16
BF16 = mybir.dt.bfloat16
F8 = mybir.dt.float8e4
U8 = mybir.dt.uint8
AF = mybir.ActivationFunctionType
OP = mybir.AluOpType
DR = mybir.MatmulPerfMode.DoubleRow

NCORES = 8
N = 4096
NL = N // NCORES          # 512 nodes per core
CIN = 1024
H = 1024
HQ = H // 2
CO = 512
R = 10
P = 128
JT = N // P               # 32 j-tiles
ET = H // P               # 8
IT = NL // P              # 4 local-node tiles
GT = CO // P              # 4
KC = CIN // P             # 8 cin k-tiles
BN_EPS = 1e-5
SM = 128.0                # fp8 scale on the adjacency block
SY1 = 4.0                 # fp8 scale on Y1
SY2 = 8.0                 # fp8 scale on Y2


def build():
    """Build the SPMD Bass graph (identical on all 8 cores)."""
    nc = bacc.Bacc(None, target_bir_lowering=False, debug=False, num_devices=NCORES)

    # ---- external parameters (per-core shards / replicated) ----
    xT_d = nc.declare_dram_parameter("xT", [CIN, NL], BF16, isOutput=False)
    wmap_d = nc.declare_dram_parameter("wmap", [CIN, H], BF16, isOutput=False)
    w1_d = nc.declare_dram_parameter("w1", [H, H], BF16, isOutput=False)
    w2_d = nc.declare_dram_parameter("w2", [H, CO], BF16, isOutput=False)
    nv1T_d = nc.declare_dram_parameter("nv1T", [R, N], BF16, isOutput=False)
    nv2s_d = nc.declare_dram_parameter("nv2s", [R, NL], BF16, isOutput=False)
    bmap_d = nc.declare_dram_parameter("bmap_t", [P, ET], F32, isOutput=False)
    b1_d = nc.declare_dram_parameter("b1_t", [P, ET], F32, isOutput=False)
    s1_d = nc.declare_dram_parameter("s1_t", [P, ET], F32, isOutput=False)
    t1_d = nc.declare_dram_parameter("t1_t", [P, ET], F32, isOutput=False)
    b2_d = nc.declare_dram_parameter("b2_t", [P, GT], F32, isOutput=False)
    s2_d = nc.declare_dram_parameter("s2_t", [P, GT], F32, isOutput=False)
    t2_d = nc.declare_dram_parameter("t2_t", [P, GT], F32, isOutput=False)
    out_d = nc.declare_dram_parameter("out", [P, GT], F32, isOutput=True)

    # ---- internal DRAM: collective bounce buffers + dinv scratch ----
    rg = [list(range(NCORES))]
    dmy_in = nc.dram_tensor("dmy_in", [1, 8], F32)
    dmy_out = nc.dram_tensor("dmy_out", [NCORES, 8], F32, addr_space="Shared")
    rs_in = nc.dram_tensor("rs_in", [P, JT], F32)
    rs_out = nc.dram_tensor("rs_out", [P, JT], F32, addr_space="Shared")
    y1_in = [nc.dram_tensor(f"y1_in{q}", [NL, HQ], F8) for q in range(2)]
    y1_out = [
        nc.dram_tensor(f"y1_out{q}", [N, HQ], F8, addr_space="Shared")
        for q in range(2)
    ]
    y2_in = nc.dram_tensor("y2_in", [NL, CO], F8)
    y2_out = nc.dram_tensor("y2_out", [N, CO], F8, addr_space="Shared")
    dv_dram = nc.dram_tensor("dv_dram", [NL], F32)

    cc_insts = []

    def collective(in_ap, out_ap, kind="AllGather", op=OP.bypass):
        cc = nc.gpsimd.collective_compute(
            kind, op, replica_groups=rg,
            ins=[in_ap], outs=[out_ap],
        )
        cc_insts.append(cc)
        return cc

    with tile.TileContext(nc) as tc:
        with ExitStack() as ctx:
            # dummy first collective: absorbs entry-barrier + cold ncfw cost
            collective(dmy_in[:], dmy_out[:])

            # ---------- persistent pool ----------
            pp = ctx.enter_context(tc.tile_pool(name="persist", bufs=1))

            # nodevec operands staged at 4 partition offsets for row-tiled
            # K=10 matmuls (tile_position=(32g, 0)); group g owns j-tiles
            # 8g..8g+7
            nv1T4 = pp.tile([P, ET * P], BF16)
            nv2s4 = pp.tile([P, NL], BF16)
            for g in range(4):
                nc.sync.dma_start(
                    nv1T4[32 * g:32 * g + R, :],
                    nv1T_d[:, g * ET * P:(g + 1) * ET * P],
                )
                nc.scalar.dma_start(nv2s4[32 * g:32 * g + R, :], nv2s_d[:])

            # per-k-tile 2D loads spread across queues
            xT_sb = pp.tile([P, KC * NL], BF16)
            wm_sb = pp.tile([P, KC * H], BF16)
            w1_sb = pp.tile([P, KC * H], BF16)
            w2_sb = pp.tile([P, ET * CO], BF16)
            for kt in range(KC):
                nc.sync.dma_start(
                    xT_sb[:, kt * NL:(kt + 1) * NL],
                    xT_d[kt * P:(kt + 1) * P, :],
                )
                nc.gpsimd.dma_start(
                    wm_sb[:, kt * H:(kt + 1) * H],
                    wmap_d[kt * P:(kt + 1) * P, :],
                )
            for kt in range(KC):
                nc.sync.dma_start(
                    w1_sb[:, kt * H:(kt + 1) * H],
                    w1_d[kt * P:(kt + 1) * P, :],
                )
                nc.gpsimd.dma_start(
                    w2_sb[:, kt * CO:(kt + 1) * CO],
                    w2_d[kt * P:(kt + 1) * P, :],
                )

            rs_part = pp.tile([P, JT], F32)
            rowsum_sb = pp.tile([P, JT], F32)
            r_sb = pp.tile([P, JT], F32)
            r_bf = pp.tile([P, JT], BF16)
            r128_sb = pp.tile([P, JT], F32)
            degs_sb = pp.tile([P, NL], F16)
            ones16 = pp.tile([P, 1], F16)
            scr_sb = pp.tile([1, 2], F32)
            dinv_loc = pp.tile([1, NL], F32)
            dinvT = pp.tile([P, IT], F32)
            dinv_rep = pp.tile([P, NL], F32)

            bmap_sb = pp.tile([P, ET], F32)
            b1_sb = pp.tile([P, ET], F32)
            s1_sb = pp.tile([P, ET], F32)
            t1_sb = pp.tile([P, ET], F32)
            b2_sb = pp.tile([P, GT], F32)
            s2_sb = pp.tile([P, GT], F32)
            t2_sb = pp.tile([P, GT], F32)
            for sb, d in (
                (bmap_sb, bmap_d), (b1_sb, b1_d), (s1_sb, s1_d), (t1_sb, t1_d),
                (b2_sb, b2_d), (s2_sb, s2_d), (t2_sb, t2_d),
            ):
                nc.scalar.dma_start(sb[:], d[:])

            M_sb = pp.tile([P, JT * NL], BF16)       # E = exp(relu(z)) block
            M8_sb = pp.tile([P, JT, NL], F8)         # fp8(SM * r_j * E)
            ident_sb = pp.tile([P, P], F32)
            ident8 = pp.tile([P, P], F8)             # SM * I
            diag8 = pp.tile([P, IT, NL], F8)         # self-loop rhs tiles
            make_identity(nc, ident_sb[:])
            nc.vector.tensor_scalar_mul(ident8[:], ident_sb[:], SM)
            nc.gpsimd.memset(diag8[:], 0.0)
            nc.gpsimd.memset(degs_sb[:], 0.0)
            nc.gpsimd.memset(ones16[:], 1.0)
            nc.gpsimd.memset(scr_sb[:], 1.0)
            for it in range(IT):
                nc.vector.tensor_copy(
                    diag8[:, it, it * P:(it + 1) * P], ident8[:]
                )

            xmT_sb = pp.tile([P, ET * NL], BF16)     # relu(x wmap)^T, [f, i]
            xw1bf_sb = pp.tile([P, IT * H], BF16)    # local XW1, [i, f]
            y1q8_sb = pp.tile([P, IT, H], F8)        # fp8(SY1 dinv XW1)
            h1T_sb = pp.tile([P, ET * NL], BF16)     # conv1 out (bn1), [f, i]
            y2q8_sb = pp.tile([P, IT, CO], F8)       # fp8(SY2 dinv XW2)
            h2tmp = pp.tile([P, NL], BF16)
            jtmp = pp.tile([P, P], BF16)
            pool_part = pp.tile([P, GT], F32)

            # exp table preload so the first real Exp skips ACT_TABLE_LOAD
            nc.scalar.activation(scr_sb[:, 0:1], scr_sb[:, 1:2], AF.Exp)

            # ---------- phase A1: z, E, rowsum AR, xmT, XW1 ----------
            with ExitStack() as a1:
                psZ = a1.enter_context(
                    tc.tile_pool(name="psZ", bufs=1, space="PSUM")
                )
                psB = a1.enter_context(
                    tc.tile_pool(name="psB", bufs=3, space="PSUM")
                )
                tmpZ = a1.enter_context(tc.tile_pool(name="tmpZ", bufs=6))

                # z = nv1 @ nv2, 4 row-groups concurrent (K=10); per tile:
                # DVE relu into fp16, scalar Exp with fused rowsum accumulate
                for rr in range(ET):
                    for g in range(4):
                        jt = ET * g + rr
                        zp = psZ.tile([P, NL], F32, tag=f"z{g}", name=f"z{jt}")
                        nc.tensor.matmul(
                            zp[:],
                            nv1T4[32 * g:32 * g + R, rr * P:(rr + 1) * P],
                            nv2s4[32 * g:32 * g + R, :],
                            start=True, stop=True,
                            tile_position=(32 * g, 0),
                        )
                        zr = tmpZ.tile([P, NL], F16, tag="zr", name=f"zr{jt}")
                        nc.vector.tensor_scalar(
                            zr[:], zp[:], 0.0, 0.0, op0=OP.max, op1=OP.add
                        )
                        nc.scalar.activation(
                            M_sb[:, jt * NL:(jt + 1) * NL], zr[:], AF.Exp,
                            accum_out=rs_part[:, jt:jt + 1],
                        )

                # ---- AllReduce softmax row-sum partials ----
                nc.sync.dma_start(rs_in[:], rs_part[:])
                collective(rs_in[:], rs_out[:], kind="AllReduce", op=OP.add)
                # rsqrt table preload during the idle window before dinv
                nc.scalar.activation(
                    scr_sb[:, 0:1], scr_sb[:, 1:2], AF.Abs_reciprocal_sqrt,
                    bias=1.0,
                )
                nc.sync.dma_start(rowsum_sb[:], rs_out[:])
                with nc.allow_low_precision(reason="r feeds a bf16 matmul"):
                    nc.vector.reciprocal(r_bf[:], rowsum_sb[:])
                nc.vector.reciprocal(r_sb[:], rowsum_sb[:])
                nc.vector.tensor_scalar_mul(r128_sb[:], r_sb[:], SM)

                # ---- xmT = relu(wmap^T x^T + b), epilogue on DVE ----
                for et in range(ET):
                    mp = psB.tile([P, NL], F32, tag="mp", name=f"mp{et}")
                    for kt in range(KC):
                        nc.tensor.matmul(
                            mp[:],
                            wm_sb[:, kt * H + et * P: kt * H + (et + 1) * P],
                            xT_sb[:, kt * NL:(kt + 1) * NL],
                            start=(kt == 0), stop=(kt == KC - 1),
                        )
                    nc.vector.tensor_scalar(
                        xmT_sb[:, et * NL:(et + 1) * NL], mp[:],
                        bmap_sb[:, et:et + 1], 0.0,
                        op0=OP.add, op1=OP.max,
                    )

                # ---- XW1 = xm @ w1 (bf16 local copy) ----
                for q in range(2):
                    for it in range(IT):
                        wp = psB.tile([P, HQ], F32, tag="mp",
                                      name=f"wp{q}{it}")
                        for kt in range(KC):
                            nc.tensor.matmul(
                                wp[:],
                                xmT_sb[:, kt * NL + it * P: kt * NL + (it + 1) * P],
                                w1_sb[:, kt * H + q * HQ: kt * H + (q + 1) * HQ],
                                start=(kt == 0), stop=(kt == KC - 1),
                            )
                        nc.vector.tensor_copy(
                            xw1bf_sb[:, it * H + q * HQ: it * H + (q + 1) * HQ],
                            wp[:],
                        )

            # ---------- phase A2: colsum -> dinv -> Y1 cast -> AGs ----------
            with ExitStack() as a2:
                psCS = a2.enter_context(
                    tc.tile_pool(name="psCS", bufs=1, space="PSUM")
                )
                psD = a2.enter_context(
                    tc.tile_pool(name="psD", bufs=1, space="PSUM")
                )

                # col-tiled colsum: group cg accumulates j-tiles 8cg..8cg+7
                # into its own bank at psum partition 32cg
                cs = {
                    cg: psCS.tile([P, NL], F32, tag=f"cs{cg}", name=f"cs{cg}")
                    for cg in range(4)
                }
                for cr in range(ET):
                    for cg in range(4):
                        jt = ET * cg + cr
                        nc.tensor.matmul(
                            cs[cg][32 * cg:32 * cg + 1, :],
                            r_bf[:, jt:jt + 1],
                            M_sb[:, jt * NL:(jt + 1) * NL],
                            start=(cr == 0), stop=(cr == ET - 1),
                            tile_position=(0, 32 * cg),
                        )
                # deg fixup in two half-partition matmuls so the second pair
                # of psum-row copies overlaps the first matmul
                dgp = psD.tile([1, NL], F32)
                for cg in range(2):
                    nc.vector.tensor_copy(
                        degs_sb[32 * cg:32 * cg + 1, :],
                        cs[cg][32 * cg:32 * cg + 1, :],
                    )
                nc.tensor.matmul(
                    dgp[:], ones16[0:64, :], degs_sb[0:64, :],
                    start=True, stop=False,
                )
                for cg in range(2, 4):
                    nc.vector.tensor_copy(
                        degs_sb[32 * cg:32 * cg + 1, :],
                        cs[cg][32 * cg:32 * cg + 1, :],
                    )
                nc.tensor.matmul(
                    dgp[:], ones16[64:P, :], degs_sb[64:P, :],
                    start=False, stop=True,
                )
                # dinv = (deg+1)^-1/2 via scalar LUT
                nc.scalar.activation(
                    dinv_loc[:], dgp[:], AF.Abs_reciprocal_sqrt, bias=1.0
                )
                # dinvT via PE transpose (4x [1,128] -> [128,1]); avoids the
                # DRAM round-trip latency on the critical path
                tp = psD.tile([P, IT], F32, tag="dvt", name="dvt")
                for it in range(IT):
                    nc.tensor.transpose(
                        tp[:, it:it + 1],
                        dinv_loc[0:1, it * P:(it + 1) * P],
                        ident_sb[0:1, 0:1],
                    )
                nc.vector.tensor_copy(dinvT[:], tp[:])
                # dinv_rep (epilogue operand, off critical path) via DRAM
                nc.scalar.dma_start(dv_dram[:], dinv_loc[:])
                nc.gpsimd.dma_start(
                    dinv_rep[:], dv_dram[None, :].to_broadcast((P, NL))
                )

                # ---- Y1 = fp8(SY1 * dinv_i * XW1) in q-half order so the
                # first AllGather triggers as early as possible ----
                bounce_insts = []
                for q in range(2):
                    for it in range(IT):
                        nc.vector.tensor_scalar(
                            y1q8_sb[:, it, q * HQ:(q + 1) * HQ],
                            xw1bf_sb[:, it * H + q * HQ: it * H + (q + 1) * HQ],
                            dinvT[:, it:it + 1], SY1,
                            op0=OP.mult, op1=OP.mult,
                        )
                        eng = nc.scalar if it % 2 == 0 else nc.sync
                        bounce_insts.append(eng.dma_start(
                            y1_in[q][it * P:(it + 1) * P, :],
                            y1q8_sb[:, it, q * HQ:(q + 1) * HQ],
                        ))
                    collective(y1_in[q][:], y1_out[q][:])

                # ---- M8 = fp8(SM * r_j * E) on DVE only (gpsimd clashes);
                # held behind the Y1 bounces so the scheduler can't run it
                # ahead of the critical dinv/cast chain ----
                last_bounce = bounce_insts[-1]
                for jt in range(JT):
                    m8 = nc.vector.tensor_scalar_mul(
                        M8_sb[:, jt, :],
                        M_sb[:, jt * NL:(jt + 1) * NL],
                        r128_sb[:, jt:jt + 1],
                    )
                    add_dep_helper(m8.ins, last_bounce.ins, True,
                                   "m8 after y1 bounces")

            # ---------- conv passes (fp8 DoubleRow) ----------
            ENGS2 = [nc.sync, nc.scalar]
            KPE = JT // 2 - 4   # tail kps run mt-major so epilogues overlap

            def conv_pass(mts, slab_pool, ps_pool, y_out, width, off_f,
                          yq8, tagp, epilogue):
                """psum[mt] = M8^T @ Ygathered + SM * Y_self, DoubleRow fp8."""
                psums = {
                    mt: ps_pool.tile([P, NL], F32, tag=f"{tagp}{mt}",
                                     name=f"{tagp}{mt}")
                    for mt in mts
                }
                # self-loop first: doesn't need the gathered slab, so it
                # runs while the AllGather is still in flight
                for mt in mts:
                    for tp in range(IT // 2):
                        nc.tensor.matmul(
                            psums[mt][:],
                            yq8[:, 2 * tp:2 * tp + 2, mt * P:(mt + 1) * P],
                            diag8[:, 2 * tp:2 * tp + 2, :],
                            start=(tp == 0), stop=False,
                            perf_mode=DR,
                        )
                # gathered slab: 32 plain 2D loads (full DMA rate); early
                # chunks on the two HWDGE queues (gpsimd SWDGE would starve
                # behind in-flight DVE 2-port ops), tail chunks add the
                # gpsimd queue for extra parallelism
                slab = slab_pool.tile([P, JT, width], F8, tag=f"sl{tagp}",
                                      name=f"sl{tagp}", bufs=1)
                for kt in range(JT):
                    if kt >= 20:
                        eng = (ENGS2 + [nc.gpsimd])[kt % 3]
                    else:
                        eng = ENGS2[kt % 2]
                    eng.dma_start(
                        slab[:, kt, :], y_out[kt * P:(kt + 1) * P, :]
                    )

                def kmm(kp, mt):
                    fo = mt * P - off_f
                    nc.tensor.matmul(
                        psums[mt][:],
                        slab[:, 2 * kp:2 * kp + 2, fo:fo + P],
                        M8_sb[:, 2 * kp:2 * kp + 2, :],
                        start=False,
                        stop=(kp == JT // 2 - 1),
                        perf_mode=DR,
                    )

                for kp in range(KPE):
                    for mt in mts:
                        kmm(kp, mt)
                # tail: mt-major so each psum finishes early and its
                # epilogue overlaps the next tile's matmuls
                for mt in mts:
                    for kp in range(KPE, JT // 2):
                        kmm(kp, mt)
                    epilogue(mt, psums[mt])

            def mk_epilogue(etmp_pool, b_sb, s_sb, t_sb, inv_scale, tagp,
                            hdst=None, pool_out=None):
                def epilogue(mt, psum):
                    ta = etmp_pool.tile([P, NL], F32, tag=f"ea{tagp}",
                                        name=f"ea{tagp}{mt}")
                    nc.vector.tensor_mul(ta[:], psum[:], dinv_rep[:])
                    tb = etmp_pool.tile([P, NL], F32, tag=f"eb{tagp}",
                                        name=f"eb{tagp}{mt}")
                    if pool_out is not None:
                        # pooled output only: fuse the row-sum into the Relu
                        # and apply the bn scale/shift on the host instead
                        nc.scalar.activation(
                            tb[:], ta[:], AF.Relu,
                            bias=b_sb[:, mt:mt + 1], scale=inv_scale,
                            accum_out=pool_out[:, mt:mt + 1],
                        )
                        return
                    nc.scalar.activation(
                        tb[:], ta[:], AF.Relu,
                        bias=b_sb[:, mt:mt + 1], scale=inv_scale,
                    )
                    nc.vector.tensor_scalar(
                        hdst[:, mt * NL:(mt + 1) * NL], tb[:],
                        s_sb[:, mt:mt + 1], t_sb[:, mt:mt + 1],
                        op0=OP.mult, op1=OP.add,
                    )
                return epilogue

            # conv1: two f-half passes, 8 PSUM banks total
            with ExitStack() as c1:
                ps1 = c1.enter_context(
                    tc.tile_pool(name="ps1", bufs=1, space="PSUM")
                )
                slab1_pool = c1.enter_context(
                    tc.tile_pool(name="slab1", bufs=1)
                )
                etmp = c1.enter_context(tc.tile_pool(name="etmp", bufs=2))
                epi1 = mk_epilogue(etmp, b1_sb, s1_sb, t1_sb,
                                   1.0 / (SM * SY1), "1", hdst=h1T_sb)
                for q in range(2):
                    conv_pass(range(4 * q, 4 * q + 4), slab1_pool, ps1,
                              y1_out[q], HQ, q * HQ, y1q8_sb, f"c1{q}", epi1)

                # XW2 = h1 @ w2 (bf16), contraction mt-outer: each k-tile
                # group becomes ready right after that h1 tile's epilogue,
                # so most of XW2 interleaves into conv1's matmul stream
                # (reuses conv1-q0's freed PSUM banks via the same tags)
                wp2 = {
                    it: ps1.tile([P, CO], F32, tag=f"c10{it}",
                                 name=f"wp2{it}")
                    for it in range(IT)
                }
                for kt in range(ET):
                    for it in range(IT):
                        nc.tensor.matmul(
                            wp2[it][:],
                            h1T_sb[:, kt * NL + it * P: kt * NL + (it + 1) * P],
                            w2_sb[:, kt * CO:(kt + 1) * CO],
                            start=(kt == 0), stop=(kt == ET - 1),
                        )
                for it in range(IT):
                    nc.vector.tensor_scalar(
                        y2q8_sb[:, it, :], wp2[it][:],
                        dinvT[:, it:it + 1], SY2,
                        op0=OP.mult, op1=OP.mult,
                    )
                    eng = nc.scalar if it % 2 == 0 else nc.sync
                    eng.dma_start(
                        y2_in[it * P:(it + 1) * P, :], y2q8_sb[:, it, :]
                    )
                collective(y2_in[:], y2_out[:])

            # conv2
            with ExitStack() as c2:
                ps2 = c2.enter_context(
                    tc.tile_pool(name="ps2", bufs=1, space="PSUM")
                )
                psJ = c2.enter_context(
                    tc.tile_pool(name="psJ", bufs=1, space="PSUM")
                )
                slab2_pool = c2.enter_context(
                    tc.tile_pool(name="slab2", bufs=1)
                )
                etmp2 = c2.enter_context(tc.tile_pool(name="etmp2", bufs=2))

                # keep-warm chain: small self-paced matmuls spanning the Y2
                # AllGather window so the PE clock doesn't re-throttle
                # before conv2's k-loop (each link: MM -> DVE read -> MM...)
                jp = psJ.tile([P, P], F32, tag="jk", name="jk")
                prev = None
                for k in range(14):
                    jm = nc.tensor.matmul(
                        jp[:], ident8[:], ident8[:], start=True, stop=True,
                        skip_group_check=True,
                    )
                    if prev is not None:
                        add_dep_helper(jm.ins, prev.ins, True, "warm pace")
                    prev = nc.vector.tensor_copy(jtmp[:], jp[:])

                epi2 = mk_epilogue(etmp2, b2_sb, s2_sb, t2_sb,
                                   1.0 / (SM * SY2), "2",
                                   pool_out=pool_part)
                conv_pass(range(GT), slab2_pool, ps2, y2_out, CO, 0,
                          y2q8_sb, "c2", epi2)

            # per-core pooled partial out; host reduces across cores
            nc.gpsimd.dma_start(out_d[:], pool_part[:])

        # pin the CC stream order: dummy, rs, y1h0, y1h1, y2
        for a, b in zip(cc_insts[1:], cc_insts[:-1]):
            add_dep_helper(a.ins, b.ins, True, "cc stream order")

    nc.compile()
    return nc


_NC_CACHE = {}


def _get_nc():
    if "nc" not in _NC_CACHE:
        _NC_CACHE["nc"] = build()
    return _NC_CACHE["nc"]


def make_in_maps(inputs):
    import ml_dtypes

    f = np.float32
    bf = ml_dtypes.bfloat16
    x = np.asarray(inputs["x"], dtype=f)
    w_map = np.asarray(inputs["w_map"], dtype=f)
    w1 = np.asarray(inputs["w1"], dtype=f)
    w2 = np.asarray(inputs["w2"], dtype=f)
    nv1 = np.asarray(inputs["nv1"], dtype=f)
    nv2 = np.asarray(inputs["nv2"], dtype=f)

    def vec_t(v, nt):
        return np.ascontiguousarray(np.asarray(v, dtype=f).reshape(nt, P).T)

    s1 = (np.asarray(inputs["bn1_g"], f)
          / np.sqrt(np.asarray(inputs["bn1_v"], f) + BN_EPS))
    t1 = np.asarray(inputs["bn1_b"], f) - np.asarray(inputs["bn1_m"], f) * s1
    s2 = (np.asarray(inputs["bn2_g"], f)
          / np.sqrt(np.asarray(inputs["bn2_v"], f) + BN_EPS))
    t2 = np.asarray(inputs["bn2_b"], f) - np.asarray(inputs["bn2_m"], f) * s2

    common = {
        "wmap": np.ascontiguousarray(w_map.astype(bf)),
        "w1": np.ascontiguousarray(w1.astype(bf)),
        "w2": np.ascontiguousarray(w2.astype(bf)),
        "nv1T": np.ascontiguousarray(nv1.T.astype(bf)),
        "bmap_t": vec_t(inputs["b_map"], ET),
        "b1_t": vec_t(inputs["b1"], ET),
        "s1_t": vec_t(s1, ET),
        "t1_t": vec_t(t1, ET),
        "b2_t": vec_t(inputs["b2"], GT),
        "s2_t": vec_t(s2, GT),
        "t2_t": vec_t(t2, GT),
    }
    in_maps = []
    for c in range(NCORES):
        m = dict(common)
        m["xT"] = np.ascontiguousarray(x[c * NL:(c + 1) * NL].T.astype(bf))
        m["nv2s"] = np.ascontiguousarray(nv2[:, c * NL:(c + 1) * NL].astype(bf))
        in_maps.append(m)
    return in_maps


def finish_host(results, inputs):
    """Sum per-core pooled partials, apply bn2 + mean + attention gate."""
    f = np.float32
    pooled_sum = np.zeros(CO, f)
    for res in results:
        arr = np.asarray(res["out"], dtype=f)      # [P, GT], g = t*P + p
        pooled_sum += arr.T.reshape(-1)
    # device pools the pre-bn2 activations; apply bn2 scale/shift here
    s2 = (np.asarray(inputs["bn2_g"], f)
          / np.sqrt(np.asarray(inputs["bn2_v"], f) + BN_EPS))
    t2 = np.asarray(inputs["bn2_b"], f) - np.asarray(inputs["bn2_m"], f) * s2
    pooled = s2 * (pooled_sum / N) + t2
    w_attn = np.asarray(inputs["w_attn"], f).reshape(-1)
    b_attn = np.asarray(inputs["b_attn"], f).reshape(-1)[0]
    z = float(pooled @ w_attn + b_attn)
    attn = 1.0 / (1.0 + np.exp(-z))
    return (pooled * attn)[None, :].astype(f)


def run(inputs, trace=False, tmpdir=None):
    nc = _get_nc()
    in_maps = make_in_maps(inputs)
    res = run_bass_kernel_spmd(
        nc, in_maps, core_ids=list(range(NCORES)), trace=trace, tmpdir=tmpdir
    )
    out = finish_host(res.results, inputs)
    return out, res


def kernel(**inputs):
    out, _ = run(inputs)
    return out
